# revision 28
# baseline (speedup 1.0000x reference)
"""Trainium2 Bass kernel: 4-layer sigmoid autoencoder forward + per-sample Jacobian.

Reference computes, per sample b:
    c1 = sig(x W1^T + b1); c2 = sig(c1 W2^T + b2); c3 = sig(c2 W3^T + b3)
    recover = c3 W4^T + b4
    Jac_b = W4 diag(s3_b) W3 diag(s2_b) W2 diag(s1_b) W1      (s = c(1-c))

Key algebraic restructure: factor through the H2=128 bottleneck:
    LT_b = (diag(s3_b) W3)^T W4^T          [H2, D]
    R_b  = diag(s2_b) W2 diag(s1_b) W1     [H2, D]
    Jac_b = LT_b^T @ R_b                   rank-128 product, 268M MACs/sample
vs the reference einsum chain's 671M MACs/sample.

Distribution: pure data parallel over batch. 8 cores x 16 samples each.
Weights replicated; all transposed layouts precomputed on host. Forward
matmuls run as float32r (full-rate fp32 variant; the sigmoid-saturation
regions make the s-vectors exquisitely sensitive to pre-activation error,
so bf16 there blows past the accuracy budget); the Jacobian-path matmuls
run in bf16 with f32 PSUM accumulate and f32 output. recover's b4 bias is
added on the host after the gather.

Shape of the implementation, driven by what the hardware traces showed:
  - forward matmuls are batch-major: lhsT is the [K, 16] activation block, so
    the per-matmul LDWEIGHTS is 16 columns instead of 128, and the moving
    operand is a full 512-wide weight slab; pre-activations are then
    PE-transposed to feature-major for the per-partition sigmoid bias and
    the next layer's lhsT;
  - inputs arrive in six mega-DMAs (so consumers wait on one queue each); a
    dummy-matmul ladder makes PE observe each queue once, and PE warmup /
    keep-warm filler matmuls cover the input-DMA window and the forward's
    PE-idle points so the HAM clock gate stays released (a cold PE runs
    everything at half clock);
  - the R/L factor matmuls of sample b+1 are emitted between the jac tiles
    of sample b (software pipelining), as [128,512] single-PSUM-bank chunks
    copied out immediately, so the in-order PE never stalls long on jac
    PSUM slots;
  - jac tiles are 2-PSUM-bank [128, 1024] blocks: one PSUM->SBUF copy
    instruction (split 3 ACT / 5 DVE per sample) and one fully-contiguous
    512KB DMA each; PSUM slots are tag-split so a slot is only ever read by
    one engine class (the WAR wait then merges with the RAW wait -- the
    self-loading matmul ISA struct has a single sync-wait slot, and extra
    waits cost event-semaphore chains).

Measured on TRN2 (neuron-profile exec_time_ns, whole NEFF): ~212us on a
warm chip (~238us when the fleet clock-throttles), vs ~186us HBM-write
floor for the 512MB Jacobian output. rel err ~5.5e-3 (gate 2e-2).
"""

import numpy as np
import ml_dtypes

import concourse.mybir as mybir
import concourse.tile as tile
from concourse import bacc
from concourse.bass_utils import run_bass_kernel_spmd
from concourse.masks import make_identity

B, D, H1, H2 = 128, 1024, 512, 128
NCORES = 8
BS = B // NCORES  # 16 samples per core

F32 = mybir.dt.float32
F32R = mybir.dt.float32r
BF16 = mybir.dt.bfloat16
AF = mybir.ActivationFunctionType
ALU = mybir.AluOpType

# wr_a1/wr_a2 (f32r): layer-1 weights (k-chunks 0-3 / 4-7), forward-critical
WA_F = 2048
# w23 (f32r): w2t [128, 4, 128] | w3t [128, 512]
W23_F = 1024
# wr_b (bf16): jacobian weights
WB_F = 8192        # w1r [128,4,1024] | w4tr [128,4,1024]
# bias/f32 block: b1c [128,4] | b2c [128,1] | b3c [128,4] | w2t_f | w3r_f
BIA_F = 9 + 512 + 512


def _p(a, pin=128):
    """[K*pin, F...] -> [pin, K, F...] partition-major layout, contiguous."""
    a = np.ascontiguousarray(a)
    k = a.shape[0] // pin
    return np.ascontiguousarray(
        a.reshape(k, pin, *a.shape[1:]).transpose(1, 0, *range(2, a.ndim + 1))
    )


def build_nc():
    nc = bacc.Bacc()

    xc_e = nc.declare_dram_parameter("xcb", [128, 8, BS], F32R, isOutput=False)
    bia_e = nc.declare_dram_parameter("bias", [128, BIA_F], F32, isOutput=False)
    wa1_e = nc.declare_dram_parameter("wr_a1", [128, WA_F], F32R, isOutput=False)
    wa2_e = nc.declare_dram_parameter("wr_a2", [128, WA_F], F32R, isOutput=False)
    w23_e = nc.declare_dram_parameter("w23", [128, W23_F], F32R, isOutput=False)
    wb_e = nc.declare_dram_parameter("wr_b", [128, WB_F], BF16, isOutput=False)
    rec_e = nc.declare_dram_parameter("recover", [BS, D], F32, isOutput=True)
    c2_e = nc.declare_dram_parameter("c2out", [BS, H2], F32, isOutput=True)
    jac_e = nc.declare_dram_parameter("jac", [BS, D, D], F32, isOutput=True)

    with tile.TileContext(nc) as tc:
        with (
            tc.tile_pool(name="w", bufs=1) as wp,
            tc.tile_pool(name="act", bufs=1) as ap,
            tc.tile_pool(name="samp", bufs=2) as sp,
            tc.tile_pool(name="jout", bufs=8) as jp,
            tc.tile_pool(name="psA", bufs=2, space="PSUM") as psf,
            tc.tile_pool(name="psja", bufs=1, space="PSUM") as psja,
            tc.tile_pool(name="psjd", bufs=2, space="PSUM") as psjd,
        ):
            IDN = wp.tile([128, 128], F32)
            make_identity(nc, IDN[:])
            XCB = wp.tile([128, 8, BS], F32R)
            nc.sync.dma_start(XCB[:], xc_e[:])
            BIA = wp.tile([128, BIA_F], F32)
            nc.sync.dma_start(BIA[:], bia_e[:])
            WA1 = wp.tile([128, WA_F], F32R)
            nc.sync.dma_start(WA1[:], wa1_e[:])
            WA2 = wp.tile([128, WA_F], F32R)
            nc.sync.dma_start(WA2[:], wa2_e[:])
            W23 = wp.tile([128, W23_F], F32R)
            nc.sync.dma_start(W23[:], w23_e[:])
            WB = wp.tile([128, WB_F], BF16)
            nc.sync.dma_start(WB[:], wb_e[:])

            W1Ta = WA1[:, 0:2048].rearrange("p (a b) -> p a b", b=512)
            W1Tb = WA2[:, 0:2048].rearrange("p (a b) -> p a b", b=512)
            W2T = W23[:, 0:512].rearrange("p (a b) -> p a b", b=128)
            W3T = W23[:, 512:1024]
            B1 = BIA[:, 0:4]
            B2 = BIA[:, 4:5]
            B3 = BIA[:, 5:9]
            W2TF = BIA[:, 9:521].rearrange("p (a b) -> p a b", b=128)
            W3RF = BIA[:, 521:1033].rearrange("p (a b) -> p a b", b=128)
            W1R = WB[:, 0:4096].rearrange("p (a b) -> p a b", b=1024)
            W4TR = WB[:, 4096:8192].rearrange("p (a b) -> p a b", b=1024)

            mm = nc.tensor.matmul

            # --- PE warmup under the input DMAs + keep-warm fillers at the
            # forward's PE-idle points: the HAM clock gate re-throttles after
            # ~3.4us of PE idleness, and a cold PE runs everything at half
            # clock. The filler tile borrows a jd-pool PSUM slot (released
            # before the jac stream needs both).
            wt = psjd.tile([128, 512], F32, tag="jd")

            def warm(n):
                for _ in range(n):
                    mm(wt[:, 0:128], IDN[:], IDN[:], start=True, stop=True)

            warm(10)
            # --- dummy ladder: PE observes each input DMA queue once.
            pd = psf.tile([2, 2], F32, tag="f")
            for src in (XCB[:, 0, 0:2], BIA[:, 0:2], WA1[:, 0:2],
                        WA2[:, 0:2], W23[:, 0:2], WB[:, 0:2]):
                mm(pd[:], src, src, start=True, stop=True)

            def scale_w2s(b):
                w2s = sp.tile([128, 4, H2], BF16, tag="w2s")
                nc.gpsimd.tensor_tensor(
                    w2s[:], W2TF[:],
                    s1T[:, :, b:b + 1].to_broadcast([128, 4, H2]), ALU.mult)
                return w2s

            def scale_w3s(b):
                w3s = sp.tile([128, 4, H2], BF16, tag="w3s")
                nc.gpsimd.tensor_tensor(
                    w3s[:], W3RF[:],
                    s3T[:, :, b:b + 1].to_broadcast([128, 4, H2]), ALU.mult)
                return w3s

            def scale_ws(b):
                return scale_w2s(b), scale_w3s(b)

            def rl_chunk(dst, wsrc, rhs, nsl, scale):
                """One [128,512] R/L chunk: 4 accumulating matmuls into a
                1-bank PSUM tile + immediate scaled copy into dst."""
                pf = psf.tile([128, 512], F32, tag="f")
                for k in range(4):
                    mm(pf[:], wsrc[:, k, :], rhs[:, k, nsl],
                       start=(k == 0), stop=(k == 3))
                if scale is None:
                    nc.scalar.copy(dst[:, nsl], pf[:])
                else:
                    nc.scalar.activation(dst[:, nsl], pf[:], AF.Copy,
                                         scale=scale)


            warm(6)

            # ---------------- forward pass (batched over 16 samples) ----------
            # Batch-major matmuls ([16, N] outputs), then PE-transpose the
            # pre-activations into feature-major [feat, 16] for the
            # per-partition sigmoid bias and the next layer's lhsT.
            def to_featT(flat, cT_mm, cT_f, bias, nchunk):
                """flat [16, 128*nchunk] f32 pre-acts -> transposed sigmoid
                outputs: cT_mm (f32r/bf16 for the next matmul) and cT_f
                (f32 for the s-vectors)."""
                tps = psf.tile([128, 16 * nchunk], F32, tag="f")
                for m in range(nchunk):
                    nc.tensor.transpose(
                        tps[:, m * 16:(m + 1) * 16],
                        flat[:, m * 128:(m + 1) * 128], IDN[0:BS, 0:BS])
                for m in range(nchunk):
                    nc.scalar.activation(
                        cT_f[:, m, :], tps[:, m * 16:(m + 1) * 16],
                        AF.Sigmoid, bias=bias[:, m:m + 1])
                nc.vector.tensor_copy(cT_mm[:], cT_f[:])

            # layer 1
            p1 = psf.tile([BS, H1], F32, tag="f")
            for k in range(8):
                w1c = W1Ta[:, k, :] if k < 4 else W1Tb[:, k - 4, :]
                mm(p1[:], XCB[:, k, :], w1c, start=(k == 0), stop=(k == 7))
            warm(4)
            c1f = ap.tile([BS, H1], F32)
            nc.vector.tensor_copy(c1f[:], p1[:])
            c1T = ap.tile([128, 4, BS], F32R)
            c1Tf = ap.tile([128, 4, BS], F32)
            to_featT(c1f, c1T, c1Tf, B1, 4)
            s1T = ap.tile([128, 4, BS], F32)
            nc.vector.tensor_tensor(s1T[:], c1Tf[:], c1Tf[:], ALU.mult)
            nc.vector.tensor_tensor(s1T[:], c1Tf[:], s1T[:], ALU.subtract)
            w2s0 = scale_w2s(0)
            Rs = sp.tile([128, D], BF16, tag="rs")
            Ls = sp.tile([128, D], BF16, tag="ls")
            warm(4)

            # layer 2
            p2 = psf.tile([BS, H2], F32, tag="f")
            for k in range(4):
                mm(p2[:], c1T[:, k, :], W2T[:, k, :], start=(k == 0),
                   stop=(k == 3))
            c2f = ap.tile([BS, H2], F32)
            nc.vector.tensor_copy(c2f[:], p2[:])
            c2T = ap.tile([128, 1, BS], F32R)
            c2Tf = ap.tile([128, 1, BS], F32)
            to_featT(c2f, c2T, c2Tf, B2, 1)
            s2T = ap.tile([128, BS], F32)
            nc.vector.tensor_tensor(s2T[:], c2Tf[:, 0, :], c2Tf[:, 0, :],
                                    ALU.mult)
            nc.vector.tensor_tensor(s2T[:], c2Tf[:, 0, :], s2T[:],
                                    ALU.subtract)
            warm(3)
            rl_chunk(Rs, w2s0, W1R, slice(0, 512), s2T[:, 0:1])
            rl_chunk(Rs, w2s0, W1R, slice(512, 1024), s2T[:, 0:1])

            # layer 3
            p3 = psf.tile([BS, H1], F32, tag="f")
            mm(p3[:], c2T[:, 0, :], W3T[:], start=True, stop=True)
            c3f = ap.tile([BS, H1], F32)
            nc.vector.tensor_copy(c3f[:], p3[:])
            c3T = ap.tile([128, 4, BS], BF16)
            c3Tf = ap.tile([128, 4, BS], F32)
            to_featT(c3f, c3T, c3Tf, B3, 4)
            s3T = ap.tile([128, 4, BS], F32)
            nc.vector.tensor_tensor(s3T[:], c3Tf[:], c3Tf[:], ALU.mult)
            nc.vector.tensor_tensor(s3T[:], c3Tf[:], s3T[:], ALU.subtract)
            w3s0 = scale_w3s(0)
            warm(2)
            rl_chunk(Ls, w3s0, W4TR, slice(0, 512), None)
            rl_chunk(Ls, w3s0, W4TR, slice(512, 1024), None)

            # recover [BS, D] = c3 W4^T (b4 added on the host) — off the
            # critical path to the first jac tile.
            recsb = ap.tile([BS, D], F32)
            for n in range(2):
                nsl = slice(n * 512, (n + 1) * 512)
                prec = psf.tile([BS, 512], F32, tag="f")
                for k in range(4):
                    mm(prec[:], c3T[:, k, :], W4TR[:, k, nsl],
                       start=(k == 0), stop=(k == 3))
                nc.scalar.copy(recsb[:, nsl], prec[:])
            nc.sync.dma_start(rec_e[:], recsb[:])

            # c2 output [BS, H2] via PE transpose of the post-sigmoid c2T
            tp = psf.tile([BS, 128], F32, tag="f")
            nc.tensor.transpose(tp[:], c2Tf[:, 0, :], IDN[:])
            c2sb = ap.tile([BS, 128], F32)
            nc.scalar.copy(c2sb[:], tp[:])
            nc.sync.dma_start(c2_e[:], c2sb[:])

            # ---------------- Jacobian (software-pipelined over samples) ------
            def chunk_list(b, Rs, Ls, w2s, w3s):
                sc = s2T[:, b:b + 1]
                return [(Rs, w2s, W1R, slice(0, 512), sc),
                        (Ls, w3s, W4TR, slice(0, 512), None),
                        (Rs, w2s, W1R, slice(512, 1024), sc),
                        (Ls, w3s, W4TR, slice(512, 1024), None)]

            ws = {1: scale_ws(1)}
            for b in range(BS):
                if b + 2 < BS:
                    ws[b + 2] = scale_ws(b + 2)
                filler = []
                if b + 1 < BS:
                    Rs1 = sp.tile([128, D], BF16, tag="rs")
                    Ls1 = sp.tile([128, D], BF16, tag="ls")
                    filler = chunk_list(b + 1, Rs1, Ls1, *ws.pop(b + 1))

                for m in range(8):
                    on_act = m in (0, 3, 6)
                    pool = psja if on_act else psjd
                    jpx = pool.tile([128, D], F32, tag="ja" if on_act else "jd")
                    for n in range(2):
                        nsl = slice(n * 512, (n + 1) * 512)
                        mm(jpx[:, nsl], Ls[:, m * 128:(m + 1) * 128],
                           Rs[:, nsl], start=True, stop=True)
                    if m < len(filler):
                        rl_chunk(*filler[m])
                    jsb = jp.tile([128, D], F32, tag="jsb")
                    if on_act:
                        nc.scalar.copy(jsb[:], jpx[:])
                    else:
                        nc.vector.tensor_copy(jsb[:], jpx[:])
                    nc.sync.dma_start(jac_e[b, m * 128:(m + 1) * 128, :], jsb[:])
                if b + 1 < BS:
                    Rs, Ls = Rs1, Ls1

    return nc


def _prep_shared(W1, W2, W3, W4, b1, b2, b3):
    """Host-packed input blocks shared by all cores."""
    bf = ml_dtypes.bfloat16
    w2t = _p(np.ascontiguousarray(W2.T)).reshape(128, -1)
    w3r = _p(W3).reshape(128, -1)
    w4t = _p(np.ascontiguousarray(W4.T)).reshape(128, -1)
    w1t = _p(np.ascontiguousarray(W1.T)).reshape(128, -1).astype(np.float32)
    wr_a1 = np.ascontiguousarray(w1t[:, :2048])
    wr_a2 = np.ascontiguousarray(w1t[:, 2048:])
    assert wr_a1.shape == (128, WA_F), wr_a1.shape
    w23 = np.concatenate([
        w2t,                                               # w2t [128, 512]
        np.ascontiguousarray(W3.T),                        # w3t [128, 512]
    ], axis=1).astype(np.float32)
    assert w23.shape == (128, W23_F), w23.shape
    wr_b = np.concatenate(
        [_p(W1).reshape(128, -1), w4t], axis=1).astype(bf)
    assert wr_b.shape == (128, WB_F), wr_b.shape
    bias = np.concatenate([
        np.ascontiguousarray(b1.reshape(4, 128).T),
        np.ascontiguousarray(b2.reshape(128, 1)),
        np.ascontiguousarray(b3.reshape(4, 128).T),
        w2t, w3r,
    ], axis=1).astype(np.float32)
    assert bias.shape == (128, BIA_F), bias.shape
    return wr_a1, wr_a2, w23, wr_b, bias


_CACHE = {}


def _get_nc():
    if "nc" not in _CACHE:
        nc = build_nc()
        if not nc.is_finalized():
            nc.finalize()
        _CACHE["nc"] = nc
    return _CACHE["nc"]


def run(x, W1, b1, W2, b2, W3, b3, W4, b4, trace=False, **spmd_kwargs):
    f = lambda a: np.ascontiguousarray(np.asarray(a, dtype=np.float32))
    x, W1, b1, W2, b2, W3, b3, W4, b4 = map(
        f, (x, W1, b1, W2, b2, W3, b3, W4, b4))
    wr_a1, wr_a2, w23, wr_b, bias = _prep_shared(W1, W2, W3, W4, b1, b2, b3)
    in_maps = []
    for i in range(NCORES):
        xs = x[i * BS:(i + 1) * BS]          # [16, 1024]
        xcb = _p(np.ascontiguousarray(xs.T))  # [128, 8, 16] f32
        in_maps.append({"xcb": xcb, "bias": bias, "wr_a1": wr_a1,
                        "wr_a2": wr_a2, "w23": w23, "wr_b": wr_b})
    res = run_bass_kernel_spmd(
        _get_nc(), in_maps, core_ids=list(range(NCORES)), trace=trace,
        **spmd_kwargs)
    recover = np.concatenate(
        [r["recover"] for r in res.results], axis=0) + b4[None, :]
    c2 = np.concatenate([r["c2out"] for r in res.results], axis=0)
    jac = np.concatenate([r["jac"] for r in res.results], axis=0)
    return (recover, c2, jac), res


def kernel(x, W1, b1, W2, b2, W3, b3, W4, b4):
    out, _ = run(x, W1, b1, W2, b2, W3, b3, W4, b4)
    return out


# revision 29
# speedup vs baseline: 1.0478x; 1.0478x over previous
"""Trainium2 Bass kernel: 4-layer sigmoid autoencoder forward + per-sample Jacobian.

Reference computes, per sample b:
    c1 = sig(x W1^T + b1); c2 = sig(c1 W2^T + b2); c3 = sig(c2 W3^T + b3)
    recover = c3 W4^T + b4
    Jac_b = W4 diag(s3_b) W3 diag(s2_b) W2 diag(s1_b) W1      (s = c(1-c))

Key algebraic restructure: factor through the H2=128 bottleneck:
    LT_b = (diag(s3_b) W3)^T W4^T          [H2, D]
    R_b  = diag(s2_b) W2 diag(s1_b) W1     [H2, D]
    Jac_b = LT_b^T @ R_b                   rank-128 product, 268M MACs/sample
vs the reference einsum chain's 671M MACs/sample.

Distribution: pure data parallel over batch. 8 cores x 16 samples each.
Weights replicated; all transposed layouts precomputed on host. Forward
matmuls run as float32r (full-rate fp32 variant; the sigmoid-saturation
regions make the s-vectors exquisitely sensitive to pre-activation error,
so bf16 there blows past the accuracy budget); the Jacobian-path matmuls
run in bf16 with f32 PSUM accumulate and f32 output. recover's b4 bias is
added on the host after the gather.

Shape of the implementation, driven by what the hardware traces showed:
  - forward matmuls are batch-major: lhsT is the [K, 16] activation block, so
    the per-matmul LDWEIGHTS is 16 columns instead of 128, and the moving
    operand is a full 512-wide weight slab; pre-activations are then
    PE-transposed to feature-major for the per-partition sigmoid bias and
    the next layer's lhsT;
  - inputs arrive in six mega-DMAs (so consumers wait on one queue each); a
    dummy-matmul ladder makes PE observe each queue once, and PE warmup /
    keep-warm filler matmuls cover the input-DMA window and the forward's
    PE-idle points so the HAM clock gate stays released (a cold PE runs
    everything at half clock);
  - the R/L factor matmuls of sample b+1 are emitted between the jac tiles
    of sample b (software pipelining), as [128,512] single-PSUM-bank chunks
    copied out immediately, so the in-order PE never stalls long on jac
    PSUM slots;
  - jac tiles are 2-PSUM-bank [128, 1024] blocks: one PSUM->SBUF copy
    instruction (split 3 ACT / 5 DVE per sample) and one fully-contiguous
    512KB DMA each; PSUM slots are tag-split so a slot is only ever read by
    one engine class (the WAR wait then merges with the RAW wait -- the
    self-loading matmul ISA struct has a single sync-wait slot, and extra
    waits cost event-semaphore chains).

Measured on TRN2 (neuron-profile exec_time_ns, whole NEFF): ~212us on a
warm chip (~238us when the fleet clock-throttles), vs ~186us HBM-write
floor for the 512MB Jacobian output. rel err ~5.5e-3 (gate 2e-2).
"""

import numpy as np
import ml_dtypes

import concourse.mybir as mybir
import concourse.tile as tile
from concourse import bacc
from concourse.bass_utils import run_bass_kernel_spmd
from concourse.masks import make_identity

B, D, H1, H2 = 128, 1024, 512, 128
NCORES = 8
BS = B // NCORES  # 16 samples per core

F32 = mybir.dt.float32
F32R = mybir.dt.float32r
BF16 = mybir.dt.bfloat16
AF = mybir.ActivationFunctionType
ALU = mybir.AluOpType

# wr_a1/wr_a2 (f32r): layer-1 weights (k-chunks 0-3 / 4-7), forward-critical
WA_F = 2048
# w23 (f32r): w2t [128, 4, 128] | w3t [128, 512]
W23_F = 1024
# wr_b (bf16): jacobian weights
WB_F = 8192        # w1r [128,4,1024] | w4tr [128,4,1024]
# bias/f32 block: b1c [128,4] | b2c [128,1] | b3c [128,4] | w2t_f | w3r_f
BIA_F = 9 + 512 + 512


def _p(a, pin=128):
    """[K*pin, F...] -> [pin, K, F...] partition-major layout, contiguous."""
    a = np.ascontiguousarray(a)
    k = a.shape[0] // pin
    return np.ascontiguousarray(
        a.reshape(k, pin, *a.shape[1:]).transpose(1, 0, *range(2, a.ndim + 1))
    )


def build_nc():
    nc = bacc.Bacc()

    xc_e = nc.declare_dram_parameter("xcb", [128, 8, BS], F32R, isOutput=False)
    bia_e = nc.declare_dram_parameter("bias", [128, BIA_F], F32, isOutput=False)
    wa1_e = nc.declare_dram_parameter("wr_a1", [128, WA_F], F32R, isOutput=False)
    wa2_e = nc.declare_dram_parameter("wr_a2", [128, WA_F], F32R, isOutput=False)
    w23_e = nc.declare_dram_parameter("w23", [128, W23_F], F32R, isOutput=False)
    wb_e = nc.declare_dram_parameter("wr_b", [128, WB_F], BF16, isOutput=False)
    rec_e = nc.declare_dram_parameter("recover", [BS, D], F32, isOutput=True)
    c2_e = nc.declare_dram_parameter("c2out", [BS, H2], F32, isOutput=True)
    jac_e = nc.declare_dram_parameter("jac", [BS, D, D], F32, isOutput=True)

    with tile.TileContext(nc) as tc:
        with (
            tc.tile_pool(name="w", bufs=1) as wp,
            tc.tile_pool(name="act", bufs=1) as ap,
            tc.tile_pool(name="samp", bufs=2) as sp,
            tc.tile_pool(name="jout", bufs=8) as jp,
            tc.tile_pool(name="psA", bufs=2, space="PSUM") as psf,
            tc.tile_pool(name="psja", bufs=1, space="PSUM") as psja,
            tc.tile_pool(name="psjd", bufs=2, space="PSUM") as psjd,
        ):
            IDN = wp.tile([128, 128], F32)
            make_identity(nc, IDN[:])
            XCB = wp.tile([128, 8, BS], F32R)
            nc.sync.dma_start(XCB[:], xc_e[:])
            WA1 = wp.tile([128, WA_F], F32R)
            nc.sync.dma_start(WA1[:], wa1_e[:])
            WA2 = wp.tile([128, WA_F], F32R)
            nc.sync.dma_start(WA2[:], wa2_e[:])
            BIA = wp.tile([128, BIA_F], F32)
            nc.sync.dma_start(BIA[:], bia_e[:])
            W23 = wp.tile([128, W23_F], F32R)
            nc.sync.dma_start(W23[:], w23_e[:])
            WB = wp.tile([128, WB_F], BF16)
            nc.sync.dma_start(WB[:], wb_e[:])

            W1Ta = WA1[:, 0:2048].rearrange("p (a b) -> p a b", b=512)
            W1Tb = WA2[:, 0:2048].rearrange("p (a b) -> p a b", b=512)
            W2T = W23[:, 0:512].rearrange("p (a b) -> p a b", b=128)
            W3T = W23[:, 512:1024]
            B1 = BIA[:, 0:4]
            B2 = BIA[:, 4:5]
            B3 = BIA[:, 5:9]
            W2TF = BIA[:, 9:521].rearrange("p (a b) -> p a b", b=128)
            W3RF = BIA[:, 521:1033].rearrange("p (a b) -> p a b", b=128)
            W1R = WB[:, 0:4096].rearrange("p (a b) -> p a b", b=1024)
            W4TR = WB[:, 4096:8192].rearrange("p (a b) -> p a b", b=1024)

            mm = nc.tensor.matmul

            # --- PE warmup under the input DMAs + keep-warm fillers at the
            # forward's PE-idle points: the HAM clock gate re-throttles after
            # ~3.4us of PE idleness, and a cold PE runs everything at half
            # clock. The filler tile borrows a jd-pool PSUM slot (released
            # before the jac stream needs both).
            wt = psjd.tile([128, 512], F32, tag="jd")

            def warm(n):
                for _ in range(n):
                    mm(wt[:, 0:128], IDN[:], IDN[:], start=True, stop=True)

            warm(10)
            # --- dummy ladder: PE observes each input DMA queue once.
            pd = psf.tile([2, 2], F32, tag="f")
            for src in (XCB[:, 0, 0:2], BIA[:, 0:2], WA1[:, 0:2],
                        WA2[:, 0:2], W23[:, 0:2], WB[:, 0:2]):
                mm(pd[:], src, src, start=True, stop=True)

            def scale_w2s(b):
                w2s = sp.tile([128, 4, H2], BF16, tag="w2s")
                nc.gpsimd.tensor_tensor(
                    w2s[:], W2TF[:],
                    s1T[:, :, b:b + 1].to_broadcast([128, 4, H2]), ALU.mult)
                return w2s

            def scale_w3s(b):
                w3s = sp.tile([128, 4, H2], BF16, tag="w3s")
                nc.gpsimd.tensor_tensor(
                    w3s[:], W3RF[:],
                    s3T[:, :, b:b + 1].to_broadcast([128, 4, H2]), ALU.mult)
                return w3s

            def scale_ws(b):
                return scale_w2s(b), scale_w3s(b)

            def rl_chunk(dst, wsrc, rhs, nsl, scale):
                """One [128,512] R/L chunk: 4 accumulating matmuls into a
                1-bank PSUM tile + immediate scaled copy into dst."""
                pf = psf.tile([128, 512], F32, tag="f")
                for k in range(4):
                    mm(pf[:], wsrc[:, k, :], rhs[:, k, nsl],
                       start=(k == 0), stop=(k == 3))
                if scale is None:
                    nc.scalar.copy(dst[:, nsl], pf[:])
                else:
                    nc.scalar.activation(dst[:, nsl], pf[:], AF.Copy,
                                         scale=scale)


            warm(6)

            # ---------------- forward pass (batched over 16 samples) ----------
            # Batch-major matmuls ([16, N] outputs), then PE-transpose the
            # pre-activations into feature-major [feat, 16] for the
            # per-partition sigmoid bias and the next layer's lhsT.
            def to_featT(flat, cT_mm, cT_f, bias, nchunk):
                """flat [16, 128*nchunk] f32 pre-acts -> transposed sigmoid
                outputs: cT_mm (f32r/bf16 for the next matmul) and cT_f
                (f32 for the s-vectors)."""
                tps = psf.tile([128, 16 * nchunk], F32, tag="f")
                for m in range(nchunk):
                    nc.tensor.transpose(
                        tps[:, m * 16:(m + 1) * 16],
                        flat[:, m * 128:(m + 1) * 128], IDN[0:BS, 0:BS])
                for m in range(nchunk):
                    nc.scalar.activation(
                        cT_f[:, m, :], tps[:, m * 16:(m + 1) * 16],
                        AF.Sigmoid, bias=bias[:, m:m + 1])
                nc.vector.tensor_copy(cT_mm[:], cT_f[:])

            # layer 1
            p1 = psf.tile([BS, H1], F32, tag="f")
            for k in range(8):
                w1c = W1Ta[:, k, :] if k < 4 else W1Tb[:, k - 4, :]
                mm(p1[:], XCB[:, k, :], w1c, start=(k == 0), stop=(k == 7))
            warm(4)
            c1f = ap.tile([BS, H1], F32)
            nc.vector.tensor_copy(c1f[:], p1[:])
            c1T = ap.tile([128, 4, BS], F32R)
            c1Tf = ap.tile([128, 4, BS], F32)
            to_featT(c1f, c1T, c1Tf, B1, 4)
            s1T = ap.tile([128, 4, BS], F32)
            nc.vector.tensor_tensor(s1T[:], c1Tf[:], c1Tf[:], ALU.mult)
            nc.vector.tensor_tensor(s1T[:], c1Tf[:], s1T[:], ALU.subtract)
            w2s0 = scale_w2s(0)
            Rs = sp.tile([128, D], BF16, tag="rs")
            Ls = sp.tile([128, D], BF16, tag="ls")
            warm(4)

            # layer 2
            p2 = psf.tile([BS, H2], F32, tag="f")
            for k in range(4):
                mm(p2[:], c1T[:, k, :], W2T[:, k, :], start=(k == 0),
                   stop=(k == 3))
            c2f = ap.tile([BS, H2], F32)
            nc.vector.tensor_copy(c2f[:], p2[:])
            c2T = ap.tile([128, 1, BS], F32R)
            c2Tf = ap.tile([128, 1, BS], F32)
            to_featT(c2f, c2T, c2Tf, B2, 1)
            s2T = ap.tile([128, BS], F32)
            nc.vector.tensor_tensor(s2T[:], c2Tf[:, 0, :], c2Tf[:, 0, :],
                                    ALU.mult)
            nc.vector.tensor_tensor(s2T[:], c2Tf[:, 0, :], s2T[:],
                                    ALU.subtract)
            warm(3)
            rl_chunk(Rs, w2s0, W1R, slice(0, 512), s2T[:, 0:1])
            rl_chunk(Rs, w2s0, W1R, slice(512, 1024), s2T[:, 0:1])

            # layer 3
            p3 = psf.tile([BS, H1], F32, tag="f")
            mm(p3[:], c2T[:, 0, :], W3T[:], start=True, stop=True)
            c3f = ap.tile([BS, H1], F32)
            nc.vector.tensor_copy(c3f[:], p3[:])
            c3T = ap.tile([128, 4, BS], BF16)
            c3Tf = ap.tile([128, 4, BS], F32)
            to_featT(c3f, c3T, c3Tf, B3, 4)
            s3T = ap.tile([128, 4, BS], F32)
            nc.vector.tensor_tensor(s3T[:], c3Tf[:], c3Tf[:], ALU.mult)
            nc.vector.tensor_tensor(s3T[:], c3Tf[:], s3T[:], ALU.subtract)
            w3s0 = scale_w3s(0)
            warm(2)
            rl_chunk(Ls, w3s0, W4TR, slice(0, 512), None)
            rl_chunk(Ls, w3s0, W4TR, slice(512, 1024), None)

            # recover [BS, D] = c3 W4^T (b4 added on the host) — off the
            # critical path to the first jac tile.
            recsb = ap.tile([BS, D], F32)
            for n in range(2):
                nsl = slice(n * 512, (n + 1) * 512)
                prec = psf.tile([BS, 512], F32, tag="f")
                for k in range(4):
                    mm(prec[:], c3T[:, k, :], W4TR[:, k, nsl],
                       start=(k == 0), stop=(k == 3))
                nc.scalar.copy(recsb[:, nsl], prec[:])
            nc.sync.dma_start(rec_e[:], recsb[:])

            # c2 output [BS, H2] via PE transpose of the post-sigmoid c2T
            tp = psf.tile([BS, 128], F32, tag="f")
            nc.tensor.transpose(tp[:], c2Tf[:, 0, :], IDN[:])
            c2sb = ap.tile([BS, 128], F32)
            nc.scalar.copy(c2sb[:], tp[:])
            nc.sync.dma_start(c2_e[:], c2sb[:])

            # ---------------- Jacobian (software-pipelined over samples) ------
            def chunk_list(b, Rs, Ls, w2s, w3s):
                sc = s2T[:, b:b + 1]
                return [(Rs, w2s, W1R, slice(0, 512), sc),
                        (Ls, w3s, W4TR, slice(0, 512), None),
                        (Rs, w2s, W1R, slice(512, 1024), sc),
                        (Ls, w3s, W4TR, slice(512, 1024), None)]

            ws = {1: scale_ws(1)}
            for b in range(BS):
                if b + 2 < BS:
                    ws[b + 2] = scale_ws(b + 2)
                filler = []
                if b + 1 < BS:
                    Rs1 = sp.tile([128, D], BF16, tag="rs")
                    Ls1 = sp.tile([128, D], BF16, tag="ls")
                    filler = chunk_list(b + 1, Rs1, Ls1, *ws.pop(b + 1))

                for m in range(8):
                    on_act = m in (0, 3, 6)
                    pool = psja if on_act else psjd
                    jpx = pool.tile([128, D], F32, tag="ja" if on_act else "jd")
                    for n in range(2):
                        nsl = slice(n * 512, (n + 1) * 512)
                        mm(jpx[:, nsl], Ls[:, m * 128:(m + 1) * 128],
                           Rs[:, nsl], start=True, stop=True)
                    if m < len(filler):
                        rl_chunk(*filler[m])
                    jsb = jp.tile([128, D], F32, tag="jsb")
                    if on_act:
                        nc.scalar.copy(jsb[:], jpx[:])
                    else:
                        nc.vector.tensor_copy(jsb[:], jpx[:])
                    nc.sync.dma_start(jac_e[b, m * 128:(m + 1) * 128, :], jsb[:])
                if b + 1 < BS:
                    Rs, Ls = Rs1, Ls1

    return nc


def _prep_shared(W1, W2, W3, W4, b1, b2, b3):
    """Host-packed input blocks shared by all cores."""
    bf = ml_dtypes.bfloat16
    w2t = _p(np.ascontiguousarray(W2.T)).reshape(128, -1)
    w3r = _p(W3).reshape(128, -1)
    w4t = _p(np.ascontiguousarray(W4.T)).reshape(128, -1)
    w1t = _p(np.ascontiguousarray(W1.T)).reshape(128, -1).astype(np.float32)
    wr_a1 = np.ascontiguousarray(w1t[:, :2048])
    wr_a2 = np.ascontiguousarray(w1t[:, 2048:])
    assert wr_a1.shape == (128, WA_F), wr_a1.shape
    w23 = np.concatenate([
        w2t,                                               # w2t [128, 512]
        np.ascontiguousarray(W3.T),                        # w3t [128, 512]
    ], axis=1).astype(np.float32)
    assert w23.shape == (128, W23_F), w23.shape
    wr_b = np.concatenate(
        [_p(W1).reshape(128, -1), w4t], axis=1).astype(bf)
    assert wr_b.shape == (128, WB_F), wr_b.shape
    bias = np.concatenate([
        np.ascontiguousarray(b1.reshape(4, 128).T),
        np.ascontiguousarray(b2.reshape(128, 1)),
        np.ascontiguousarray(b3.reshape(4, 128).T),
        w2t, w3r,
    ], axis=1).astype(np.float32)
    assert bias.shape == (128, BIA_F), bias.shape
    return wr_a1, wr_a2, w23, wr_b, bias


_CACHE = {}


def _get_nc():
    if "nc" not in _CACHE:
        nc = build_nc()
        if not nc.is_finalized():
            nc.finalize()
        _CACHE["nc"] = nc
    return _CACHE["nc"]


def run(x, W1, b1, W2, b2, W3, b3, W4, b4, trace=False, **spmd_kwargs):
    f = lambda a: np.ascontiguousarray(np.asarray(a, dtype=np.float32))
    x, W1, b1, W2, b2, W3, b3, W4, b4 = map(
        f, (x, W1, b1, W2, b2, W3, b3, W4, b4))
    wr_a1, wr_a2, w23, wr_b, bias = _prep_shared(W1, W2, W3, W4, b1, b2, b3)
    in_maps = []
    for i in range(NCORES):
        xs = x[i * BS:(i + 1) * BS]          # [16, 1024]
        xcb = _p(np.ascontiguousarray(xs.T))  # [128, 8, 16] f32
        in_maps.append({"xcb": xcb, "bias": bias, "wr_a1": wr_a1,
                        "wr_a2": wr_a2, "w23": w23, "wr_b": wr_b})
    res = run_bass_kernel_spmd(
        _get_nc(), in_maps, core_ids=list(range(NCORES)), trace=trace,
        **spmd_kwargs)
    recover = np.concatenate(
        [r["recover"] for r in res.results], axis=0) + b4[None, :]
    c2 = np.concatenate([r["c2out"] for r in res.results], axis=0)
    jac = np.concatenate([r["jac"] for r in res.results], axis=0)
    return (recover, c2, jac), res


def kernel(x, W1, b1, W2, b2, W3, b3, W4, b4):
    out, _ = run(x, W1, b1, W2, b2, W3, b3, W4, b4)
    return out
